# revision 5
# baseline (speedup 1.0000x reference)
"""Paged-attention decode kernel for 8 TRN2 NeuronCores.

Sharding: tensor-parallel over the 8 KV heads (one per core). Each core holds
its own 128-wide slice of the paged KV cache (converted to bf16), computes the
4 GQA query heads of its group for all 32 requests, and writes a [128, 128]
output block ([32 req x 4 heads, 128 dim]). The host applies the KV-cache
scatter update, builds per-core pools/indices/masks, and concatenates the 8
per-core outputs into the full [32, 32, 128] result. No collectives needed.

Device algorithm per core:
  - dma_gather(transpose=True) over block rows of the bf16 K pool yields the
    K^T layout [d=128, slot, block] directly (one gather per 4-request group).
  - QK matmuls use a zero-padded stationary q so request b's scores land on
    PSUM partitions 4b..4b+3; all 32 requests accumulate into one
    [128, 2048] PSUM scores region ([req*4+head, slot*128+block]).
  - Batched masked softmax over the full [128, 2048] region (mask from
    context_lens, built host-side).
  - 16 PE transposes produce p^T; dma_gather(transpose=False) yields V blocks
    [block, slot*128+d]; PV matmuls contract over blocks per (request, slot),
    accumulating [4, 128] per request in PSUM.
"""

import os
import sys

import numpy as np
import ml_dtypes

if "/opt/trn_rl_repo" not in sys.path:
    sys.path.insert(0, "/opt/trn_rl_repo")

import concourse.bacc as bacc
import concourse.bass as bass
import concourse.mybir as mybir
import concourse.tile as tile

BF16 = ml_dtypes.bfloat16

SCALE = 0.08838834764831845  # 1/sqrt(128)
B = 32               # requests
KVH = 8              # kv heads == cores
NH = 4               # q heads per kv head (GQA group)
DH = 128             # head dim
BS = 16              # tokens per cache block
NBLOCKS = 4096       # pool blocks
MBS = 128            # max blocks per sequence
S = MBS * BS         # 2048 max context
GROUPS = 8           # request groups per core
GR = B // GROUPS     # 4 requests per group
NIDX = GR * MBS      # 512 gathered blocks per group
NEG = -1.0e30


def build_core_program():
    """Build the single-core Bass program (same on all 8 cores)."""
    nc = bacc.Bacc("TRN2", target_bir_lowering=False)
    f32 = mybir.dt.float32
    bf16 = mybir.dt.bfloat16
    i16 = mybir.dt.int16

    k_pool = nc.dram_tensor("k_pool", [NBLOCKS, BS * DH], bf16, kind="ExternalInput")
    v_pool = nc.dram_tensor("v_pool", [NBLOCKS, BS * DH], bf16, kind="ExternalInput")
    qpad = nc.dram_tensor("qpad", [DH, B * 128], bf16, kind="ExternalInput")
    maskd = nc.dram_tensor("mask", [128, S], f32, kind="ExternalInput")
    idxd = nc.dram_tensor("idx", [128, GROUPS * (NIDX // 16)], i16, kind="ExternalInput")
    ident = nc.dram_tensor("ident", [128, 128], bf16, kind="ExternalInput")
    out = nc.dram_tensor("out", [128, DH], f32, kind="ExternalOutput")

    Exp = mybir.ActivationFunctionType.Exp
    ICOLS = NIDX // 16  # 32 idx columns per group

    with tile.TileContext(nc) as tc:
        with (
            tc.tile_pool(name="const", bufs=1) as cpool,
            tc.tile_pool(name="soft", bufs=1) as spool,
            tc.tile_pool(name="kt", bufs=2) as ktpool,
            tc.tile_pool(name="vv", bufs=2) as vpool,
            tc.tile_pool(name="outs", bufs=4) as ospool,
        ):
            qpad_sb = cpool.tile([DH, B * 128], bf16)
            mask_sb = cpool.tile([128, S], f32)
            idx_sb = cpool.tile([128, GROUPS * ICOLS], i16)
            id_sb = cpool.tile([128, 128], bf16)
            nc.sync.dma_start(qpad_sb[:], qpad[:])
            nc.sync.dma_start(mask_sb[:], maskd[:])
            nc.sync.dma_start(idx_sb[:], idxd[:])
            nc.sync.dma_start(id_sb[:], ident[:])

            s_sb = spool.tile([128, S], f32)
            p_sb = spool.tile([128, S], bf16)
            p2_sb = spool.tile([128, S], bf16)
            pt_sb = spool.tile([128, S], bf16)
            mx = spool.tile([128, 1], f32)
            negm = spool.tile([128, 1], f32)
            sums = spool.tile([128, 1], f32)
            recip = spool.tile([128, 1], f32)

            # ---- Phase B: K gathers + QK matmuls into one PSUM scores region
            NMM = (BS + 3) // 4  # N<=512 chunks of up-to-4 slots each
            with tc.tile_pool(name="pscore", bufs=1, space="PSUM") as pspool:
                scores = pspool.tile([128, S], f32)
                for g in range(GROUPS):
                    kt = ktpool.tile([128, BS, NIDX], bf16, tag="kt")
                    nc.gpsimd.dma_gather(
                        kt[:],
                        k_pool[:],
                        idx_sb[:, g * ICOLS:(g + 1) * ICOLS],
                        NIDX,
                        NIDX,
                        BS * DH,
                        transpose=True,
                    )
                    for r in range(GR):
                        b = GR * g + r
                        for mm in range(NMM):
                            nsl = min(4, BS - mm * 4)
                            nc.tensor.matmul(
                                scores[:, mm * 512: mm * 512 + nsl * 128],
                                lhsT=qpad_sb[:, b * 128:(b + 1) * 128],
                                rhs=kt[:, mm * 4: mm * 4 + nsl, r * 128:(r + 1) * 128],
                                start=(b == 0),
                                stop=(b == B - 1),
                            )

                # ---- Phase C: batched masked softmax
                nc.vector.tensor_tensor(
                    out=s_sb[:], in0=scores[:], in1=mask_sb[:], op=mybir.AluOpType.add
                )
            nc.vector.reduce_max(mx[:], s_sb[:], axis=mybir.AxisListType.X)
            nc.scalar.mul(negm[:], mx[:], -1.0)
            nc.scalar.activation(
                p_sb[:], s_sb[:], Exp, bias=negm[:, 0:1], scale=1.0,
                accum_out=sums[:, 0:1],
            )
            nc.vector.reciprocal(recip[:], sums[:])
            nc.vector.tensor_scalar_mul(p2_sb[:], p_sb[:], recip[:, 0:1])

            # ---- Phase D: p^T via PE transposes
            with tc.tile_pool(name="ptr", bufs=2, space="PSUM") as tppool:
                for cc in range(BS):
                    tp = tppool.tile([128, 128], bf16, tag="tp")
                    nc.tensor.transpose(tp[:], p2_sb[:, cc * 128:(cc + 1) * 128], id_sb[:])
                    if cc % 2 == 0:
                        nc.vector.tensor_copy(pt_sb[:, cc * 128:(cc + 1) * 128], tp[:])
                    else:
                        nc.scalar.copy(pt_sb[:, cc * 128:(cc + 1) * 128], tp[:])

            # ---- Phase E: V gathers + PV matmuls
            with tc.tile_pool(name="pout", bufs=4, space="PSUM") as popool:
                for g in range(GROUPS):
                    vt = vpool.tile([128, GR, BS * DH], bf16, tag="vt")
                    nc.gpsimd.dma_gather(
                        vt[:],
                        v_pool[:],
                        idx_sb[:, g * ICOLS:(g + 1) * ICOLS],
                        NIDX,
                        NIDX,
                        BS * DH,
                        transpose=False,
                    )
                    for r in range(GR):
                        b = GR * g + r
                        po = popool.tile([NH, DH], mybir.dt.float32, tag="po")
                        for sl in range(BS):
                            nc.tensor.matmul(
                                po[:],
                                lhsT=pt_sb[:, sl * 128 + NH * b: sl * 128 + NH * b + NH],
                                rhs=vt[:, r, sl * DH:(sl + 1) * DH],
                                start=(sl == 0),
                                stop=(sl == BS - 1),
                            )
                        os_t = ospool.tile([NH, DH], mybir.dt.float32, tag="os")
                        nc.vector.tensor_copy(os_t[:], po[:])
                        nc.sync.dma_start(out[NH * b: NH * b + NH, :], os_t[:])

    nc.compile()
    return nc


def _host_inputs(q, k, v, k_cache, v_cache, slot_mapping, block_tables, context_lens):
    """Apply the scatter update and build per-core input dicts."""
    D = KVH * DH
    kc = np.asarray(k_cache, dtype=np.float32).reshape(NBLOCKS * BS, D).copy()
    vc = np.asarray(v_cache, dtype=np.float32).reshape(NBLOCKS * BS, D).copy()
    slot = np.asarray(slot_mapping, dtype=np.int64)
    keep = slot >= 0
    kc[slot[keep]] = np.asarray(k, dtype=np.float32).reshape(B, D)[keep]
    vc[slot[keep]] = np.asarray(v, dtype=np.float32).reshape(B, D)[keep]
    kc = kc.reshape(NBLOCKS, BS, KVH, DH)
    vc = vc.reshape(NBLOCKS, BS, KVH, DH)

    bt = np.asarray(block_tables, dtype=np.int64)
    ctx = np.asarray(context_lens, dtype=np.int64)
    qf = np.asarray(q, dtype=np.float32)

    # idx tile: per group g, 512 block ids (requests 4g..4g+3 concatenated),
    # wrapped: linear i = s*16 + p -> [p, s]; replicated to 128 partitions.
    idx = np.zeros((128, GROUPS * (NIDX // 16)), dtype=np.int16)
    for g in range(GROUPS):
        ids = bt[GR * g:GR * (g + 1)].reshape(NIDX).astype(np.int16)
        w = ids.reshape(NIDX // 16, 16).T  # [16, 32]
        idx[:, g * (NIDX // 16):(g + 1) * (NIDX // 16)] = np.tile(w, (8, 1))

    # mask [128, 2048]: row 4b+h, col sl*128 + j -> position j*16+sl
    j = np.arange(MBS)
    sl = np.arange(BS)
    pos = (j[None, :] * BS + sl[:, None]).reshape(S)  # col -> seq position
    valid = pos[None, :] < ctx[:, None]  # [B, S]
    mask_rows = np.where(valid, 0.0, NEG).astype(np.float32)  # [B, S]
    mask = np.repeat(mask_rows, NH, axis=0)  # [128, S]

    ident = np.eye(128, dtype=np.float32).astype(BF16)

    in_maps = []
    for kh in range(KVH):
        k_pool = np.ascontiguousarray(
            kc[:, :, kh, :].reshape(NBLOCKS, BS * DH)).astype(BF16)
        v_pool = np.ascontiguousarray(
            vc[:, :, kh, :].reshape(NBLOCKS, BS * DH)).astype(BF16)
        qpad = np.zeros((DH, B * 128), dtype=np.float32)
        for b in range(B):
            # stationary cols 4b..4b+3 of slice b hold q^T * SCALE
            qpad[:, b * 128 + NH * b: b * 128 + NH * b + NH] = (
                qf[b, NH * kh: NH * (kh + 1), :].T * SCALE
            )
        in_maps.append({
            "k_pool": k_pool,
            "v_pool": v_pool,
            "qpad": qpad.astype(BF16),
            "mask": mask,
            "idx": idx,
            "ident": ident,
        })
    return in_maps


def kernel(q, k, v, k_cache, v_cache, slot_mapping, block_tables, context_lens):
    from concourse.bass_utils import run_bass_kernel_spmd

    nc = build_core_program()
    in_maps = _host_inputs(
        q, k, v, k_cache, v_cache, slot_mapping, block_tables, context_lens
    )
    core_ids = list(range(KVH))
    res = run_bass_kernel_spmd(
        nc, in_maps, core_ids,
        trace=bool(int(os.environ.get("KERNEL_TRACE", "0"))),
        tmpdir=os.environ.get("KERNEL_TMPDIR") or None,
    )
    kernel.last_results = res
    outs = res.results
    full = np.empty((B, KVH * NH, DH), dtype=np.float32)
    for kh in range(KVH):
        oc = np.asarray(outs[kh]["out"], dtype=np.float32)  # [128, 128]
        full[:, NH * kh: NH * (kh + 1), :] = oc.reshape(B, NH, DH)
    return full


# revision 11
# speedup vs baseline: 1.1345x; 1.1345x over previous
"""Paged-attention decode kernel for 8 TRN2 NeuronCores.

Sharding: tensor-parallel over the 8 KV heads (one per core). Each core holds
its own 128-wide slice of the paged KV cache (converted to bf16), computes the
4 GQA query heads of its group for all 32 requests, and writes a [128, 128]
output block ([32 req x 4 heads, 128 dim]). The host applies the KV-cache
scatter update, builds per-core pools/indices/masks, and concatenates the 8
per-core outputs into the full [32, 32, 128] result. No collectives needed.

Device algorithm per core:
  - dma_gather(transpose=True) over block rows of the bf16 K pool yields the
    K^T layout [d=128, slot, block] directly (one gather per 4-request group).
  - QK matmuls use a zero-padded stationary q so request b's scores land on
    PSUM partitions 4b..4b+3; all 32 requests accumulate into one
    [128, 2048] PSUM scores region ([req*4+head, slot*128+block]).
  - Batched masked softmax over the full [128, 2048] region (mask from
    context_lens, built host-side).
  - 16 PE transposes produce p^T; dma_gather(transpose=False) yields V blocks
    [block, slot*128+d]; PV matmuls contract over blocks per (request, slot),
    accumulating [4, 128] per request in PSUM.
"""

import os
import sys

import numpy as np
import ml_dtypes

if "/opt/trn_rl_repo" not in sys.path:
    sys.path.insert(0, "/opt/trn_rl_repo")

import concourse.bacc as bacc
import concourse.bass as bass
import concourse.mybir as mybir
import concourse.tile as tile

BF16 = ml_dtypes.bfloat16

SCALE = 0.08838834764831845  # 1/sqrt(128)
B = 32               # requests
KVH = 8              # kv heads == cores
NH = 4               # q heads per kv head (GQA group)
DH = 128             # head dim
BS = 16              # tokens per cache block
NBLOCKS = 4096       # pool blocks
MBS = 128            # max blocks per sequence
S = MBS * BS         # 2048 max context
GROUPS = 8           # request groups per core
GR = B // GROUPS     # 4 requests per group
NIDX = GR * MBS      # 512 gathered blocks per group
NEG = -1.0e30


NQUEUES = 1


def build_core_program():
    """Build the single-core Bass program (same on all 8 cores)."""
    nc = bacc.Bacc(
        "TRN2", target_bir_lowering=False, num_swdge_queues=NQUEUES,
    )
    f32 = mybir.dt.float32
    bf16 = mybir.dt.bfloat16
    i16 = mybir.dt.int16

    k_pool = nc.dram_tensor("k_pool", [NBLOCKS, BS * DH], bf16, kind="ExternalInput")
    v_pool = nc.dram_tensor("v_pool", [NBLOCKS, BS * DH], bf16, kind="ExternalInput")
    qpad = nc.dram_tensor("qpad", [DH, B * 128], bf16, kind="ExternalInput")
    maskd = nc.dram_tensor("mask", [128, S], f32, kind="ExternalInput")
    idxd = nc.dram_tensor("idx", [128, GROUPS * (NIDX // 16)], i16, kind="ExternalInput")
    ident = nc.dram_tensor("ident", [128, 128], bf16, kind="ExternalInput")
    out = nc.dram_tensor("out", [128, DH], f32, kind="ExternalOutput")

    Exp = mybir.ActivationFunctionType.Exp
    ICOLS = NIDX // 16  # 32 idx columns per group

    with tile.TileContext(nc) as tc:
        with (
            tc.tile_pool(name="const", bufs=1) as cpool,
            tc.tile_pool(name="soft", bufs=1) as spool,
            tc.tile_pool(name="kt", bufs=3) as ktpool,
            tc.tile_pool(name="vv", bufs=3) as vpool,
            tc.tile_pool(name="outs", bufs=4) as ospool,
        ):
            qpad_sb = cpool.tile([DH, B * 128], bf16)
            mask_sb = cpool.tile([128, S], f32)
            idx_sb = cpool.tile([128, GROUPS * ICOLS], i16)
            id_sb = cpool.tile([128, 128], bf16)
            nc.sync.dma_start(idx_sb[:], idxd[:])
            nc.sync.dma_start(qpad_sb[:], qpad[:])
            nc.sync.dma_start(mask_sb[:], maskd[:])
            nc.sync.dma_start(id_sb[:], ident[:])

            s_sb = spool.tile([128, S], f32)
            p_sb = spool.tile([128, S], bf16)
            p2_sb = spool.tile([128, S], bf16)
            pt_sb = spool.tile([128, S], bf16)
            mx = spool.tile([128, 1], f32)
            negm = spool.tile([128, 1], f32)
            sums = spool.tile([128, 1], f32)
            recip = spool.tile([128, 1], f32)

            # ---- Phase B: K gathers + QK matmuls into one PSUM scores region
            NMM = (BS + 3) // 4  # N<=512 chunks of up-to-4 slots each
            with tc.tile_pool(name="pscore", bufs=1, space="PSUM") as pspool:
                scores = pspool.tile([128, S], f32)
                for g in range(GROUPS):
                    kt = ktpool.tile([128, BS, NIDX], bf16, tag="kt")
                    nc.gpsimd.dma_gather(
                        kt[:],
                        k_pool[:],
                        idx_sb[:, g * ICOLS:(g + 1) * ICOLS],
                        NIDX,
                        NIDX,
                        BS * DH,
                        transpose=True,
                        queue_num=g % NQUEUES,
                    )
                    for r in range(GR):
                        b = GR * g + r
                        for mm in range(NMM):
                            nsl = min(4, BS - mm * 4)
                            nc.tensor.matmul(
                                scores[:, mm * 512: mm * 512 + nsl * 128],
                                lhsT=qpad_sb[:, b * 128:(b + 1) * 128],
                                rhs=kt[:, mm * 4: mm * 4 + nsl, r * 128:(r + 1) * 128],
                                start=(b == 0),
                                stop=(b == B - 1),
                            )

                # ---- Phase C: batched masked softmax
                nc.vector.tensor_tensor(
                    out=s_sb[:], in0=scores[:], in1=mask_sb[:], op=mybir.AluOpType.add
                )
            nc.vector.reduce_max(mx[:], s_sb[:], axis=mybir.AxisListType.X)
            nc.scalar.mul(negm[:], mx[:], -1.0)
            nc.scalar.activation(
                p_sb[:], s_sb[:], Exp, bias=negm[:, 0:1], scale=1.0,
                accum_out=sums[:, 0:1],
            )
            nc.vector.reciprocal(recip[:], sums[:])
            nc.vector.tensor_scalar_mul(p2_sb[:], p_sb[:], recip[:, 0:1])

            # ---- Phase D: p^T via PE transposes
            with tc.tile_pool(name="ptr", bufs=2, space="PSUM") as tppool:
                for cc in range(BS):
                    tp = tppool.tile([128, 128], bf16, tag="tp")
                    nc.tensor.transpose(tp[:], p2_sb[:, cc * 128:(cc + 1) * 128], id_sb[:])
                    if cc % 2 == 0:
                        nc.vector.tensor_copy(pt_sb[:, cc * 128:(cc + 1) * 128], tp[:])
                    else:
                        nc.scalar.copy(pt_sb[:, cc * 128:(cc + 1) * 128], tp[:])

            # ---- Phase E: V gathers + PV matmuls
            with tc.tile_pool(name="pout", bufs=4, space="PSUM") as popool:
                for g in range(GROUPS):
                    vt = vpool.tile([128, GR, BS * DH], bf16, tag="vt")
                    nc.gpsimd.dma_gather(
                        vt[:],
                        v_pool[:],
                        idx_sb[:, g * ICOLS:(g + 1) * ICOLS],
                        NIDX,
                        NIDX,
                        BS * DH,
                        transpose=False,
                        queue_num=g % NQUEUES,
                    )
                    for r in range(GR):
                        b = GR * g + r
                        po = popool.tile([NH, DH], mybir.dt.float32, tag="po")
                        for sl in range(BS):
                            nc.tensor.matmul(
                                po[:],
                                lhsT=pt_sb[:, sl * 128 + NH * b: sl * 128 + NH * b + NH],
                                rhs=vt[:, r, sl * DH:(sl + 1) * DH],
                                start=(sl == 0),
                                stop=(sl == BS - 1),
                            )
                        os_t = ospool.tile([NH, DH], mybir.dt.float32, tag="os")
                        nc.vector.tensor_copy(os_t[:], po[:])
                        nc.sync.dma_start(out[NH * b: NH * b + NH, :], os_t[:])

    nc.compile()
    return nc


def _host_inputs(q, k, v, k_cache, v_cache, slot_mapping, block_tables, context_lens):
    """Apply the scatter update and build per-core input dicts."""
    D = KVH * DH
    kc = np.asarray(k_cache, dtype=np.float32).reshape(NBLOCKS * BS, D).copy()
    vc = np.asarray(v_cache, dtype=np.float32).reshape(NBLOCKS * BS, D).copy()
    slot = np.asarray(slot_mapping, dtype=np.int64)
    keep = slot >= 0
    kc[slot[keep]] = np.asarray(k, dtype=np.float32).reshape(B, D)[keep]
    vc[slot[keep]] = np.asarray(v, dtype=np.float32).reshape(B, D)[keep]
    kc = kc.reshape(NBLOCKS, BS, KVH, DH)
    vc = vc.reshape(NBLOCKS, BS, KVH, DH)

    bt = np.asarray(block_tables, dtype=np.int64)
    ctx = np.asarray(context_lens, dtype=np.int64)
    qf = np.asarray(q, dtype=np.float32)

    # idx tile: per group g, 512 block ids (requests 4g..4g+3 concatenated),
    # wrapped: linear i = s*16 + p -> [p, s]; replicated to 128 partitions.
    idx = np.zeros((128, GROUPS * (NIDX // 16)), dtype=np.int16)
    for g in range(GROUPS):
        ids = bt[GR * g:GR * (g + 1)].reshape(NIDX).astype(np.int16)
        w = ids.reshape(NIDX // 16, 16).T  # [16, 32]
        idx[:, g * (NIDX // 16):(g + 1) * (NIDX // 16)] = np.tile(w, (8, 1))

    # mask [128, 2048]: row 4b+h, col sl*128 + j -> position j*16+sl
    j = np.arange(MBS)
    sl = np.arange(BS)
    pos = (j[None, :] * BS + sl[:, None]).reshape(S)  # col -> seq position
    valid = pos[None, :] < ctx[:, None]  # [B, S]
    mask_rows = np.where(valid, 0.0, NEG).astype(np.float32)  # [B, S]
    mask = np.repeat(mask_rows, NH, axis=0)  # [128, S]

    ident = np.eye(128, dtype=np.float32).astype(BF16)

    in_maps = []
    for kh in range(KVH):
        k_pool = np.ascontiguousarray(
            kc[:, :, kh, :].reshape(NBLOCKS, BS * DH)).astype(BF16)
        v_pool = np.ascontiguousarray(
            vc[:, :, kh, :].reshape(NBLOCKS, BS * DH)).astype(BF16)
        qpad = np.zeros((DH, B * 128), dtype=np.float32)
        for b in range(B):
            # stationary cols 4b..4b+3 of slice b hold q^T * SCALE
            qpad[:, b * 128 + NH * b: b * 128 + NH * b + NH] = (
                qf[b, NH * kh: NH * (kh + 1), :].T * SCALE
            )
        in_maps.append({
            "k_pool": k_pool,
            "v_pool": v_pool,
            "qpad": qpad.astype(BF16),
            "mask": mask,
            "idx": idx,
            "ident": ident,
        })
    return in_maps


def kernel(q, k, v, k_cache, v_cache, slot_mapping, block_tables, context_lens):
    from concourse.bass_utils import run_bass_kernel_spmd

    nc = build_core_program()
    in_maps = _host_inputs(
        q, k, v, k_cache, v_cache, slot_mapping, block_tables, context_lens
    )
    core_ids = list(range(KVH))
    res = run_bass_kernel_spmd(
        nc, in_maps, core_ids,
        trace=bool(int(os.environ.get("KERNEL_TRACE", "0"))),
        tmpdir=os.environ.get("KERNEL_TMPDIR") or None,
    )
    kernel.last_results = res
    outs = res.results
    full = np.empty((B, KVH * NH, DH), dtype=np.float32)
    for kh in range(KVH):
        oc = np.asarray(outs[kh]["out"], dtype=np.float32)  # [128, 128]
        full[:, NH * kh: NH * (kh + 1), :] = oc.reshape(B, NH, DH)
    return full
